# revision 1
# baseline (speedup 1.0000x reference)
"""Trainium2 Bass kernel for DetectPeaks (sliding-window NMS + top-2).

Computes, for xcorr [32, 3, 64, 8192] f32:
    x = |xcorr|
    smax = sliding max over time, window 301 (centered, clipped)
    scores = where(smax == x, x, 0)
    top2 values + indices along time  -> ([32,3,64,2] f32, [32,3,64,2] int32)

Strategy: flatten to 6144 independent rows, shard 768 rows per core across
8 cores (data parallel, no communication).  Per 128-row tile:
  - abs on the scalar engine (in place, in a -1.0-padded buffer)
  - van Herk / Gil-Werman sliding max: per-301-block prefix/suffix max scans
    (tensor_tensor_scan with op=max on DVE), then
    smax[t] = max(S[t], P[t+300])
  - scores' = x + 1e30*(x - smax): exactly x at peaks (x==smax), very
    negative otherwise, so top-k of scores' == top-k of the reference's
    masked scores (for rows with >= 2 peaks; random data has ~27 peaks/row)
  - top-8 values + indices per row via DVE max / max_index, keep 2
"""

import numpy as np

NB, NC, NX, NT = 32, 3, 64, 8192
KERNEL = 301
PAD = KERNEL // 2  # 150
B = KERNEL  # van Herk block size
NBLK = 29  # ceil((PAD + NT + PAD) / B) -> cover xp coords [0, 8491]
LPAD = NBLK * B  # 8729
N_CORES = 8
ROWS = NB * NC * NX  # 6144
ROWS_PER_CORE = ROWS // N_CORES  # 768
P_DIM = 128
NTILE = ROWS_PER_CORE // P_DIM  # 6
BMC = 64  # scores block size for the two-level top-k
NBM = NT // BMC  # 128 block maxes per row

_cached = None


def _build(rows_per_core=ROWS_PER_CORE):
    import concourse.mybir as mybir
    from concourse.bacc import Bacc
    from concourse.tile import TileContext

    f32 = mybir.dt.float32
    Alu = mybir.AluOpType
    n_tiles = rows_per_core // P_DIM

    # Bacc (not plain Bass): its finalize() runs generate_event_semaphores,
    # which splits multi-sem waits into EventSemaphore prefixes — TRN2
    # instructions only have a single wait slot.
    nc = Bacc(None, target_bir_lowering=False)
    x_in = nc.dram_tensor("x", [rows_per_core, NT], f32, kind="ExternalInput")
    out_vals = nc.dram_tensor("out_vals", [rows_per_core, 8], f32, kind="ExternalOutput")
    out_idx = nc.dram_tensor("out_idx", [rows_per_core, 8], mybir.dt.uint32, kind="ExternalOutput")

    # Half-resolution (parity) van Herk: the expensive segmented scans run
    # on h[v] = max(x[2v], x[2v+1]) with window 150 / block 150, then the
    # full-res sliding max is reassembled per parity:
    #   smax[2u]   = max(H150[u],   xp[2u+300])
    #   smax[2u+1] = max(xp[2u+1],  H150[u+1])
    # with H150[v] = max(h[v..v+149]) = max(Sh[v], Ph[v+149]).
    LP2 = LPAD + 1          # 8730, even
    HLEN = LP2 // 2         # 4365
    B2 = 150
    HPAD = 30 * B2          # 4500
    MH = NT // 2 + 1        # 4097 H150 values needed

    with TileContext(nc) as tc:
        with (
            tc.tile_pool(name="const", bufs=1) as cpool,
            tc.tile_pool(name="big", bufs=2) as bigpool,
            tc.tile_pool(name="scan", bufs=1) as scanpool,
            tc.tile_pool(name="sc", bufs=1) as scpool,
            tc.tile_pool(name="small", bufs=2) as smallpool,
        ):
            # Segment mask for block-restarting max scans over h: zeros at
            # multiples of 150 (scan state = max(G2[v]*state, h[v]) restarts
            # at every 0 since all data >= 0). G2[1:] reversed provides the
            # restart markers for the reversed (suffix) scan.
            G2 = cpool.tile([P_DIM, HPAD + 1], f32, tag="G2")
            nc.vector.memset(G2[:, :], 1.0)
            nc.vector.memset(G2[:, 0:HPAD + 1:B2], 0.0)

            for i in range(n_tiles):
                rows = slice(i * P_DIM, (i + 1) * P_DIM)
                xp = bigpool.tile([P_DIM, LP2], f32, tag="xp")
                interior = xp[:, PAD:PAD + NT]
                # Pads + abs all on the scalar engine (|0|=0 keeps pads valid);
                # pads only matter as neutral (<= data) elements.  Tile 0 is
                # fully on the critical path (nothing to overlap with), so
                # chunk its DMA+abs to let compute start sooner.
                nchunk = 4 if i == 0 else 1
                CH = NT // nchunk
                for c in range(nchunk):
                    sl = slice(PAD + c * CH, PAD + (c + 1) * CH)
                    nc.sync.dma_start(xp[:, sl], x_in[rows, c * CH:(c + 1) * CH])
                    nc.scalar.activation(
                        xp[:, sl], xp[:, sl], mybir.ActivationFunctionType.Abs
                    )
                nc.scalar.memzero(xp[:, 0:PAD])
                nc.scalar.memzero(xp[:, PAD + NT:LP2])

                h = scanpool.tile([P_DIM, HPAD], f32, tag="h")
                nc.vector.tensor_tensor(
                    out=h[:, 0:HLEN], in0=xp[:, 0:LP2:2], in1=xp[:, 1:LP2:2],
                    op=Alu.max,
                )

                # Trimmed scan ranges: Ph is only read on [149, 4246) and Sh
                # on [0, 4097) (all within real h data, so no tail memset).
                PHE = B2 - 1 + MH  # 4246
                SHE = (NT // 2 // B2) * B2 + B2 - 1  # 4199, end of Sh's block
                Sh = scanpool.tile([P_DIM, HPAD], f32, tag="Sh")
                Ph = scanpool.tile([P_DIM, HPAD], f32, tag="Ph")
                nc.vector.tensor_tensor_scan(
                    Ph[:, 0:PHE], G2[:, 0:PHE], h[:, 0:PHE], 0.0,
                    op0=Alu.mult, op1=Alu.max,
                )
                nc.vector.tensor_tensor_scan(
                    Sh[:, SHE::-1], G2[:, 1:SHE + 2][:, ::-1], h[:, SHE::-1], 0.0,
                    op0=Alu.mult, op1=Alu.max,
                )

                # H150[v] = max(Sh[v], Ph[v+149]), v in [0, 4097)
                mh = scanpool.tile([P_DIM, MH], f32, tag="mh")
                nc.vector.tensor_tensor(
                    out=mh[:, :], in0=Sh[:, 0:MH], in1=Ph[:, B2 - 1:B2 - 1 + MH],
                    op=Alu.max,
                )
                # reassemble full-res smax into m (even/odd interleaved)
                m = scpool.tile([P_DIM, NT], f32, tag="m")
                nc.vector.tensor_tensor(
                    out=m[:, 0:NT:2], in0=mh[:, 0:NT // 2],
                    in1=xp[:, 2 * PAD:2 * PAD + NT:2], op=Alu.max,
                )
                nc.vector.tensor_tensor(
                    out=m[:, 1:NT:2], in0=xp[:, 1:NT:2], in1=mh[:, 1:NT // 2 + 1],
                    op=Alu.max,
                )
                # All-DVE tail: cross-engine handoffs (Pool TT) measured slower
                # end-to-end than keeping the chain on DVE (pipeline stalls).
                # In-place on m frees a full-width buffer -> xp double-buffers.
                # m <- (x >= smax) peak mask
                nc.vector.tensor_tensor(out=m, in0=interior, in1=m, op=Alu.is_ge)
                # m <- mask * x (exactly x at peaks, 0 elsewhere)
                nc.vector.tensor_tensor(out=m, in0=m, in1=interior, op=Alu.mult)

                v8 = smallpool.tile([P_DIM, 8], f32, tag="v8")
                i8 = smallpool.tile([P_DIM, 8], mybir.dt.uint32, tag="i8")
                nc.vector.max(out=v8, in_=m)
                nc.vector.max_index(out=i8, in_max=v8, in_values=m)
                nc.sync.dma_start(out_vals[rows, :], v8)
                nc.sync.dma_start(out_idx[rows, :], i8)
    return nc


def _get_module():
    global _cached
    if _cached is None:
        _cached = _build()
        # run_bass_via_pjrt serializes the module as-is; Bacc.finalize()
        # runs register allocation + event-semaphore legalization.
        _cached.finalize()
    return _cached


def run(xcorr: np.ndarray, trace: bool = False, **spmd_kwargs):
    from concourse.bass_utils import run_bass_kernel_spmd

    x = np.ascontiguousarray(np.asarray(xcorr, dtype=np.float32).reshape(ROWS, NT))
    nc = _get_module()
    in_maps = [
        {"x": x[c * ROWS_PER_CORE:(c + 1) * ROWS_PER_CORE]} for c in range(N_CORES)
    ]
    res = run_bass_kernel_spmd(
        nc, in_maps, core_ids=list(range(N_CORES)), trace=trace, **spmd_kwargs
    )
    vals = np.concatenate([r["out_vals"][:, :2] for r in res.results], axis=0)
    idx = np.concatenate([r["out_idx"][:, :2] for r in res.results], axis=0)
    topk_score = vals.reshape(NB, NC, NX, 2).astype(np.float32)
    topk_idx = idx.reshape(NB, NC, NX, 2).astype(np.int32)
    return (topk_score, topk_idx), res


def kernel(xcorr: np.ndarray, nlag=None, **_unused):
    out, _ = run(xcorr)
    return out

